# revision 19
# baseline (speedup 1.0000x reference)
"""Cumulative LayerNorm (cLN) Trainium2 Bass kernel.

x: [B=8, C=512, T=16000] fp32.  Per (b, t):
    mean[t] = cumsum_t(sum_c x) / (C*(t+1))
    var[t]  = cumsum_t(sum_c (x - mean[t'])^2) / (C*(t+1))
    out     = (x - mean) / sqrt(var + eps) * gamma + beta

Expansion used on-device (exact in real arithmetic):
    sum_c (x[c,t'] - mean[t'])^2 = ssq[t'] - 2*mean[t']*s1[t'] + C*mean[t']^2

Sharding: data-parallel over batch, one batch per NeuronCore (8 cores).

Per-core pipeline, software-pipelined at chunk granularity (5 chunks of 3200):
while chunk cc is normalized + stored, chunk cc+1 runs stats + scan and
chunk cc+2 streams in.  The host pre-shuffles x to [128, 4, T] (p-major) so
every load/store is ONE big descriptor-friendly DMA per half-chunk.

  Stats:  channel reduction via PE matmuls (f32r, 1 cyc/row) with an all-ones
          stationary column; squares on ACT (f32r out); s1/ssq PSUM rows
          [2, 400] evacuated by single ACT copies into a [2, 3200] row pair,
          then reshaped by one small DMA per stat into the compact per-chunk
          scan layout [128, 25] (t_local = p*25 + f).
  Scan:   per-partition prefix sums via DVE tensor_tensor_scan; cross-
          partition carry via a strict-lower-triangular PE matmul; cross-
          chunk carry kept in SBUF (per-chunk grand total via a tiny PE
          reduction, accumulated with a [1,2] DVE add).
  Norm:   inv/nminv rows DMA-reshaped to [1, 2*1600] and replicated across
          partitions by GPSIMD partition_broadcast; the normalization is two
          DVE passes fully in place in the x tiles:
            pass 1: (x*gamma)*inv      (scalar_tensor_tensor)
            pass 2: (nminv*gamma+beta)+.  (custom-DVE affine_then_add)
          so the gamma/beta affine costs nothing extra.

DMA issue is split across both HWDGE queues: SP carries loads + the small
reshape rows, ACT carries the stores, so a store waiting on the normalize
never head-of-line-blocks the next chunk's stat rows.
"""

import numpy as np

B, C, T = 8, 512, 16000
P = 128
NCH = C // P        # 4 channel groups
# Variable t-chunks (each divisible by 256 so halves split on a 128 multiple):
# small first chunk = short pipeline-fill latency to the first store; small
# last chunk = short drain tail after the final scan.
CHUNKS = [640, 2944, 2944, 2944, 2944, 2944, 640]
NCC = len(CHUNKS)
OFFS = [sum(CHUNKS[:i]) for i in range(NCC)]      # t offsets
F2S = [c // P for c in CHUNKS]                    # scan free dims (5/23)
F2OFF = [sum(F2S[:i]) for i in range(NCC)]        # recip row offsets
F2SUM = sum(F2S)                                  # 125
F2MAX = max(F2S)
# PSUM-row block per chunk: >=256 (full-rate f32r) and <=512 (one 2KB bank)
KBS = [320 if c == 640 else 368 for c in CHUNKS]
EPS = 1e-8

_PROGS = {}


def _build_program(trivial_affine=True):
    from contextlib import ExitStack

    import concourse.bass as bass
    import concourse.tile as tile
    from concourse import bacc, mybir

    f32 = mybir.dt.float32
    f32r = mybir.dt.float32r
    Alu = mybir.AluOpType
    Act = mybir.ActivationFunctionType

    nc = bacc.Bacc("TRN2", debug=False)
    xr = nc.dram_tensor("x", [P, NCH, T], f32r, kind="ExternalInput").ap()
    lstrict = nc.dram_tensor("lstrict", [P, P], f32, kind="ExternalInput").ap()
    recip5 = nc.dram_tensor("recip5", [P, F2SUM], f32, kind="ExternalInput").ap()
    gamma_pc = nc.dram_tensor("gamma_pc", [P, NCH], f32, kind="ExternalInput").ap()
    beta_pc = nc.dram_tensor("beta_pc", [P, NCH], f32, kind="ExternalInput").ap()
    out = nc.dram_tensor("out", [P, NCH, T], f32, kind="ExternalOutput").ap()

    with tile.TileContext(nc) as tc:
        with ExitStack() as ctx:
            singles = ctx.enter_context(tc.tile_pool(name="singles", bufs=1))
            xhp = ctx.enter_context(tc.tile_pool(name="xhp", bufs=5))
            xsqp = ctx.enter_context(tc.tile_pool(name="xsqp", bufs=3))
            srowp = ctx.enter_context(tc.tile_pool(name="srowp", bufs=1))
            s1sqp = ctx.enter_context(tc.tile_pool(name="s1sqp", bufs=2))
            statp = ctx.enter_context(tc.tile_pool(name="statp", bufs=2))
            browp = ctx.enter_context(tc.tile_pool(name="browp", bufs=1))
            bcp = ctx.enter_context(tc.tile_pool(name="bcp", bufs=2))
            ps_stat = ctx.enter_context(
                tc.tile_pool(name="ps_stat", bufs=6, space="PSUM")
            )
            ps_c1 = ctx.enter_context(tc.tile_pool(name="ps_c1", bufs=1, space="PSUM"))
            ps_c2 = ctx.enter_context(tc.tile_pool(name="ps_c2", bufs=1, space="PSUM"))

            # ---- constants ----
            ones_col = singles.tile([P, 1], f32)
            nc.vector.memset(ones_col, 1.0)
            ones_row = singles.tile([1, P], f32)
            nc.vector.memset(ones_row, 1.0)
            ones_scan = singles.tile([P, F2MAX], f32)
            nc.vector.memset(ones_scan, 1.0)
            lstrict_sb = singles.tile([P, P], f32)
            nc.sync.dma_start(lstrict_sb, lstrict)
            recip_sb = singles.tile([P, F2SUM], f32)
            nc.sync.dma_start(recip_sb, recip5)
            gamma_sb = singles.tile([P, NCH], f32)
            nc.sync.dma_start(gamma_sb, gamma_pc)
            beta_sb = singles.tile([P, NCH], f32)
            nc.sync.dma_start(beta_sb, beta_pc)
            eps_sb = singles.tile([P, 1], f32)
            nc.vector.memset(eps_sb, EPS)
            # running grand totals of (s1, r) over completed chunks, in SBUF
            gtot = singles.tile([1, 2], f32)

            xh = {}  # half index (cc, hh) -> tile [P, NCH, HB_cc]

            def load_half(cc, hh):
                hb = CHUNKS[cc] // 2
                t0 = OFFS[cc] + hh * hb
                xt = xhp.tile([P, NCH, hb], f32r, tag="xh", name=f"xh_{cc}_{hh}")
                nc.sync.dma_start(xt, xr[:, :, t0 : t0 + hb])
                xh[(cc, hh)] = xt

            def stats(cc):
                """Channel reductions for chunk cc.  Emits ACT squares, PE
                matmuls and ACT sq-row evacs inline; returns the DVE s1-row
                evac closures + the reshape-DMA closure for interleaving, and
                the scan-input tile."""
                ccs, kb, f2 = CHUNKS[cc], KBS[cc], F2S[cc]
                nkb_h = (ccs // 2) // kb
                srow = srowp.tile([33, ccs], f32, tag="srow", name=f"srow_{cc}")
                s1sq = s1sqp.tile([P, 2, f2], f32, tag="s1sq", name=f"s1sq_{cc}")
                s1_evacs = []
                for hh in range(2):
                    xt = xh[(cc, hh)]
                    xtf = xt.bitcast(f32)
                    for k in range(nkb_h):
                        kc = hh * nkb_h + k
                        ksl = slice(k * kb, (k + 1) * kb)
                        xsq = xsqp.tile(
                            [P, NCH, kb], f32r, tag="xsq", name=f"xsq_{cc}_{kc}"
                        )
                        for j in range(NCH):
                            nc.scalar.square(xsq[:, j, :], xtf[:, j, ksl])
                        s1p = ps_stat.tile([1, kb], f32, tag="st", name=f"s1p_{cc}_{kc}")
                        sqp = ps_stat.tile([1, kb], f32, tag="st", name=f"sqp_{cc}_{kc}")
                        for j in range(NCH):
                            nc.tensor.matmul(
                                s1p,
                                ones_col.bitcast(f32r),
                                xt[:, j, ksl],
                                start=(j == 0),
                                stop=(j == NCH - 1),
                            )
                        for j in range(NCH):
                            nc.tensor.matmul(
                                sqp,
                                ones_col.bitcast(f32r),
                                xsq[:, j, :],
                                start=(j == 0),
                                stop=(j == NCH - 1),
                            )
                        ksl2 = slice(kc * kb, (kc + 1) * kb)
                        nc.scalar.copy(srow[32:33, ksl2], sqp)
                        s1_evacs.append(
                            lambda ksl2=ksl2, s1p=s1p, srow=srow: nc.vector.tensor_copy(
                                srow[0:1, ksl2], s1p
                            )
                        )

                def reshape():
                    nc.sync.dma_start(s1sq[:, 0, :], srow[0:1, :])
                    nc.sync.dma_start(s1sq[:, 1, :], srow[32:33, :])

                return s1_evacs, reshape, s1sq

            def scan_steps(cc, s1sq):
                """Prefix-scan stats for chunk cc as a list of step closures
                (interleaved between normalize ops by the caller).
                Returns (steps, invnm tile)."""
                f2 = F2S[cc]
                s1c = s1sq[:, 0, :]
                sqc = s1sq[:, 1, :]
                rc = recip_sb[:, F2OFF[cc] : F2OFF[cc] + f2]
                osc = ones_scan[:, 0:f2]
                cum1 = statp.tile([P, f2], f32, tag="cum1", name=f"cum1_{cc}")
                carry1 = ps_c1.tile([P, 1], f32, tag="c1", name=f"c1_{cc}")
                carry1_sb = statp.tile([P, 1], f32, tag="cs1", name=f"cs1_{cc}")
                mean_c = statp.tile([P, f2], f32, tag="mean", name=f"mean_{cc}")
                u_c = statp.tile([P, f2], f32, tag="u", name=f"u_{cc}")
                cumr = statp.tile([P, f2], f32, tag="cumr", name=f"cumr_{cc}")
                carry2 = ps_c2.tile([P, 1], f32, tag="c2", name=f"c2_{cc}")
                carry2_sb = statp.tile([P, 1], f32, tag="cs2", name=f"cs2_{cc}")
                var_c = statp.tile([P, f2], f32, tag="var", name=f"var_{cc}")
                std_c = statp.tile([P, f2], f32, tag="std", name=f"std_{cc}")
                invnm = statp.tile([P, 2, f2], f32, tag="invnm", name=f"invnm_{cc}")
                last = cc == NCC - 1
                tot = (
                    None
                    if last
                    else ps_stat.tile([1, 2], f32, tag="st", name=f"tot_{cc}")
                )

                def s0():
                    nc.vector.tensor_tensor_scan(
                        cum1, osc, s1c, 0.0, Alu.mult, Alu.add
                    )
                    if cc > 0:
                        nc.tensor.matmul(
                            carry1, ones_row, gtot[0:1, 0:1], start=True, stop=False
                        )
                    nc.tensor.matmul(
                        carry1,
                        lstrict_sb,
                        cum1[:, f2 - 1 : f2],
                        start=(cc == 0),
                        stop=True,
                    )
                    if not last:
                        nc.tensor.matmul(
                            tot[0:1, 0:1],
                            ones_col,
                            cum1[:, f2 - 1 : f2],
                            start=True,
                            stop=True,
                        )

                def s1():
                    nc.vector.tensor_copy(carry1_sb, carry1)

                def s2():
                    nc.vector.scalar_tensor_tensor(
                        mean_c, cum1, carry1_sb, rc, Alu.add, Alu.mult
                    )

                def s3():
                    nc.vector.scalar_tensor_tensor(
                        u_c, mean_c, -float(C) / 2.0, s1c, Alu.mult, Alu.add
                    )
                    nc.vector.tensor_mul(u_c, mean_c, u_c)

                def s4():
                    nc.vector.scalar_tensor_tensor(
                        u_c, u_c, -2.0, sqc, Alu.mult, Alu.add
                    )

                def s5():
                    nc.vector.tensor_tensor_scan(
                        cumr, osc, u_c, 0.0, Alu.mult, Alu.add
                    )
                    if cc > 0:
                        nc.tensor.matmul(
                            carry2, ones_row, gtot[0:1, 1:2], start=True, stop=False
                        )
                    nc.tensor.matmul(
                        carry2,
                        lstrict_sb,
                        cumr[:, f2 - 1 : f2],
                        start=(cc == 0),
                        stop=True,
                    )
                    if not last:
                        nc.tensor.matmul(
                            tot[0:1, 1:2],
                            ones_col,
                            cumr[:, f2 - 1 : f2],
                            start=True,
                            stop=True,
                        )

                def s6():
                    nc.vector.tensor_copy(carry2_sb, carry2)

                def s7():
                    nc.vector.scalar_tensor_tensor(
                        var_c, cumr, carry2_sb, rc, Alu.add, Alu.mult
                    )
                    nc.scalar.activation(std_c, var_c, Act.Sqrt, bias=eps_sb)

                def s8():
                    if last:
                        return
                    if cc == 0:
                        nc.vector.tensor_copy(gtot, tot)
                    else:
                        tot_sb = statp.tile([1, 2], f32, tag="tsb", name=f"tsb_{cc}")
                        nc.vector.tensor_copy(tot_sb, tot)
                        nc.vector.tensor_add(gtot, gtot, tot_sb)

                def s9():
                    nc.vector.reciprocal(invnm[:, 0, :], std_c)

                def s10():
                    nc.vector.scalar_tensor_tensor(
                        invnm[:, 1, :], mean_c, -1.0, invnm[:, 0, :], Alu.mult, Alu.mult
                    )

                return [s0, s1, s2, s3, s4, s5, s6, s7, s8, s9, s10], invnm

            def make_bc(cc, hh, invnm):
                """inv/nminv rows for half hh -> broadcast tile [P, 2, HB]."""
                hb = CHUNKS[cc] // 2
                np_h = hb // F2S[cc]  # partitions per half in the scan layout
                brow = browp.tile([1, 2, hb], f32, tag="brow", name=f"brow_{cc}_{hh}")
                psl = slice(np_h * hh, np_h * (hh + 1))
                nc.sync.dma_start(brow[:, 0, :], invnm[psl, 0, :])
                nc.sync.dma_start(brow[:, 1, :], invnm[psl, 1, :])
                bc = bcp.tile([P, 2, hb], f32, tag="bc", name=f"bc_{cc}_{hh}")
                nc.gpsimd.partition_broadcast(bc, brow)
                return bc

            def norm_ops(cc, bcs):
                """Normalize closures for chunk cc, in place in the x tiles.
                Fast path (gamma==1, beta==0): the channel-group dim is fused
                into one op via a stride-0 broadcast AP, 8 ops per chunk.
                General path: per-group mul + fused affine_then_add, 16 ops."""
                hb = CHUNKS[cc] // 2
                ops = []
                for hh in range(2):
                    xtf = xh[(cc, hh)].bitcast(f32)
                    bc = bcs[hh]
                    if trivial_affine:
                        for q in range(2):
                            qsl = slice(q * (hb // 2), (q + 1) * (hb // 2))
                            xq = xtf[:, :, qsl]
                            inv_b, _ = bass.broadcast_tensor_aps(
                                bc[:, 0:1, qsl], xq
                            )
                            nm_b, _ = bass.broadcast_tensor_aps(
                                bc[:, 1:2, qsl], xq
                            )

                            def mul(xq=xq, inv_b=inv_b):
                                nc.vector.tensor_mul(xq, xq, inv_b)

                            def add(xq=xq, nm_b=nm_b):
                                nc.vector.tensor_add(xq, xq, nm_b)

                            ops.append(mul)
                            ops.append(add)
                    else:
                        for j in range(NCH):
                            xj = xtf[:, j, :]

                            def mul(xj=xj, bc=bc, j=j):
                                nc.vector.scalar_tensor_tensor(
                                    xj,
                                    xj,
                                    gamma_sb[:, j : j + 1],
                                    bc[:, 0, :],
                                    Alu.mult,
                                    Alu.mult,
                                )

                            def add(xj=xj, bc=bc, j=j):
                                nc.vector.affine_then_add(
                                    xj,
                                    bc[:, 1, :],
                                    xj,
                                    scale=gamma_sb[:, j : j + 1],
                                    bias=beta_sb[:, j : j + 1],
                                )

                            ops.append(mul)
                            ops.append(add)
                return ops

            def store(cc, hh):
                hb = CHUNKS[cc] // 2
                t0 = OFFS[cc] + hh * hb
                xtf = xh[(cc, hh)].bitcast(f32)
                nc.sync.dma_start(out[:, :, t0 : t0 + hb], xtf)

            # ---- prologue: chunks 0,1 and half of 2 in flight; full
            # stats+scan+bc chain for chunk 0 (nothing to overlap with yet)
            halves = [(cc, hh) for cc in range(NCC) for hh in range(2)]
            nld = 0
            for _ in range(5):
                load_half(*halves[nld])
                nld += 1
            ev0, rs0, s1sq0 = stats(0)
            for e in ev0:
                e()
            rs0()
            steps0, invnm0 = scan_steps(0, s1sq0)
            for st in steps0:
                st()
            bcs = [make_bc(0, hh, invnm0) for hh in range(2)]

            # ---- software-pipelined bodies: normalize/store chunk cc while
            # chunk cc+1 runs stats+scan and chunk cc+2 streams in
            for cc in range(NCC):
                for _ in range(2):
                    if nld < len(halves):
                        load_half(*halves[nld])
                        nld += 1
                N = norm_ops(cc, bcs)
                nh = len(N) // 2  # ops per half
                if cc + 1 < NCC:
                    evacs, reshape, s1sq_n = stats(cc + 1)
                    S, invnm_n = scan_steps(cc + 1, s1sq_n)
                    # interleave: s1 evacs ride the first normalize ops, the
                    # scan chain rides the rest so its serial latency hides
                    # under normalize throughput.
                    h0, h1 = N[:nh], N[nh:]
                    ne = max(1, (len(evacs) + len(h0) - 2) // max(1, len(h0) - 1))
                    ei = 0
                    for i, op in enumerate(h0):
                        op()
                        while ei < len(evacs) and ei < (i + 1) * ne:
                            evacs[ei]()
                            ei += 1
                    while ei < len(evacs):
                        evacs[ei]()
                        ei += 1
                    reshape()
                    # scan groups between the h1 normalize ops
                    groups = [
                        (S[0],),
                        (S[1], S[2]),
                        (S[3], S[4]),
                        (S[5],),
                        (S[6], S[7]),
                        (S[8],),
                        (S[9], S[10]),
                    ]
                    gi = 0
                    for i, op in enumerate(h1):
                        if i == len(h1) - 1:
                            break
                        op()
                        take = max(1, (len(groups) - gi) // (len(h1) - 1 - i))
                        for _ in range(take):
                            if gi < len(groups):
                                for st in groups[gi]:
                                    st()
                                gi += 1
                    while gi < len(groups):
                        for st in groups[gi]:
                            st()
                        gi += 1
                    bc0 = make_bc(cc + 1, 0, invnm_n)
                    bc1 = make_bc(cc + 1, 1, invnm_n)
                    store(cc, 0)
                    h1[-1]()
                    store(cc, 1)
                    bcs = [bc0, bc1]
                else:
                    for op in N[:nh]:
                        op()
                    store(cc, 0)
                    for op in N[nh:]:
                        op()
                    store(cc, 1)

    nc.finalize()
    return nc


def _make_consts():
    cols = []
    for i in range(NCC):
        f2 = F2S[i]
        t = OFFS[i] + np.arange(CHUNKS[i], dtype=np.float64).reshape(P, f2)
        cols.append(1.0 / (C * (t + 1.0)))
    recip5 = np.ascontiguousarray(np.concatenate(cols, axis=1).astype(np.float32))
    lstrict = np.triu(np.ones((P, P), dtype=np.float32), k=1)
    return lstrict, recip5


def _make_in_map(xb, gamma, beta):
    """Per-core input dict. xb: [C, T] fp32; gamma/beta: [C]."""
    lstrict, recip5 = _make_consts()
    return {
        "x": np.ascontiguousarray(xb.reshape(NCH, P, T).transpose(1, 0, 2)),
        "lstrict": lstrict,
        "recip5": recip5,
        "gamma_pc": np.ascontiguousarray(gamma.reshape(NCH, P).T),
        "beta_pc": np.ascontiguousarray(beta.reshape(NCH, P).T),
    }


def _from_out_layout(o):
    """Device out [P, NCH, T] -> [C, T]."""
    return np.ascontiguousarray(o.transpose(1, 0, 2).reshape(C, T))


def kernel(x, gamma, beta):
    from concourse import bass_utils

    x = np.ascontiguousarray(np.asarray(x, dtype=np.float32))
    gamma = np.asarray(gamma, dtype=np.float32).reshape(C)
    beta = np.asarray(beta, dtype=np.float32).reshape(C)

    trivial = bool(np.all(gamma == 1.0) and np.all(beta == 0.0))
    if trivial not in _PROGS:
        _PROGS[trivial] = _build_program(trivial_affine=trivial)
    prog = _PROGS[trivial]

    in_maps = [_make_in_map(x[b], gamma, beta) for b in range(B)]
    res = bass_utils.run_bass_kernel_spmd(prog, in_maps, core_ids=list(range(B)))
    return np.stack(
        [_from_out_layout(res.results[b]["out"]) for b in range(B)], axis=0
    )


# revision 20
# speedup vs baseline: 1.0122x; 1.0122x over previous
"""Cumulative LayerNorm (cLN) Trainium2 Bass kernel.

x: [B=8, C=512, T=16000] fp32.  Per (b, t):
    mean[t] = cumsum_t(sum_c x) / (C*(t+1))
    var[t]  = cumsum_t(sum_c (x - mean[t'])^2) / (C*(t+1))
    out     = (x - mean) / sqrt(var + eps) * gamma + beta

Expansion used on-device (exact in real arithmetic):
    sum_c (x[c,t'] - mean[t'])^2 = ssq[t'] - 2*mean[t']*s1[t'] + C*mean[t']^2

Sharding: data-parallel over batch, one batch per NeuronCore (8 cores).

Per-core pipeline, software-pipelined at chunk granularity (5 chunks of 3200):
while chunk cc is normalized + stored, chunk cc+1 runs stats + scan and
chunk cc+2 streams in.  The host pre-shuffles x to [128, 4, T] (p-major) so
every load/store is ONE big descriptor-friendly DMA per half-chunk.

  Stats:  channel reduction via PE matmuls (f32r, 1 cyc/row) with an all-ones
          stationary column; squares on ACT (f32r out); s1/ssq PSUM rows
          [2, 400] evacuated by single ACT copies into a [2, 3200] row pair,
          then reshaped by one small DMA per stat into the compact per-chunk
          scan layout [128, 25] (t_local = p*25 + f).
  Scan:   per-partition prefix sums via DVE tensor_tensor_scan; cross-
          partition carry via a strict-lower-triangular PE matmul; cross-
          chunk carry kept in SBUF (per-chunk grand total via a tiny PE
          reduction, accumulated with a [1,2] DVE add).
  Norm:   inv/nminv rows DMA-reshaped to [1, 2*1600] and replicated across
          partitions by GPSIMD partition_broadcast; the normalization is two
          DVE passes fully in place in the x tiles:
            pass 1: (x*gamma)*inv      (scalar_tensor_tensor)
            pass 2: (nminv*gamma+beta)+.  (custom-DVE affine_then_add)
          so the gamma/beta affine costs nothing extra.

DMA issue is split across both HWDGE queues: SP carries loads + the small
reshape rows, ACT carries the stores, so a store waiting on the normalize
never head-of-line-blocks the next chunk's stat rows.
"""

import numpy as np

B, C, T = 8, 512, 16000
P = 128
NCH = C // P        # 4 channel groups
# Variable t-chunks (each divisible by 256 so halves split on a 128 multiple):
# small first chunk = short pipeline-fill latency to the first store; small
# last chunk = short drain tail after the final scan.
CHUNKS = [640, 2944, 2944, 2944, 2944, 2944, 640]
NCC = len(CHUNKS)
OFFS = [sum(CHUNKS[:i]) for i in range(NCC)]      # t offsets
F2S = [c // P for c in CHUNKS]                    # scan free dims (5/23)
F2OFF = [sum(F2S[:i]) for i in range(NCC)]        # recip row offsets
F2SUM = sum(F2S)                                  # 125
F2MAX = max(F2S)
# PSUM-row block per chunk: >=256 (full-rate f32r) and <=512 (one 2KB bank)
KBS = [320 if c == 640 else 368 for c in CHUNKS]
EPS = 1e-8

_PROGS = {}


def _build_program(trivial_affine=True):
    from contextlib import ExitStack

    import concourse.bass as bass
    import concourse.tile as tile
    from concourse import bacc, mybir

    f32 = mybir.dt.float32
    f32r = mybir.dt.float32r
    Alu = mybir.AluOpType
    Act = mybir.ActivationFunctionType

    nc = bacc.Bacc("TRN2", debug=False)
    xr = nc.dram_tensor("x", [P, NCH, T], f32r, kind="ExternalInput").ap()
    lstrict = nc.dram_tensor("lstrict", [P, P], f32, kind="ExternalInput").ap()
    recip5 = nc.dram_tensor("recip5", [P, F2SUM], f32, kind="ExternalInput").ap()
    gamma_pc = nc.dram_tensor("gamma_pc", [P, NCH], f32, kind="ExternalInput").ap()
    beta_pc = nc.dram_tensor("beta_pc", [P, NCH], f32, kind="ExternalInput").ap()
    out = nc.dram_tensor("out", [P, NCH, T], f32, kind="ExternalOutput").ap()

    with tile.TileContext(nc) as tc:
        with ExitStack() as ctx:
            singles = ctx.enter_context(tc.tile_pool(name="singles", bufs=1))
            xhp = ctx.enter_context(tc.tile_pool(name="xhp", bufs=6))
            xsqp = ctx.enter_context(tc.tile_pool(name="xsqp", bufs=2))
            srowp = ctx.enter_context(tc.tile_pool(name="srowp", bufs=1))
            s1sqp = ctx.enter_context(tc.tile_pool(name="s1sqp", bufs=2))
            statp = ctx.enter_context(tc.tile_pool(name="statp", bufs=2))
            browp = ctx.enter_context(tc.tile_pool(name="browp", bufs=2))
            bcp = ctx.enter_context(tc.tile_pool(name="bcp", bufs=4))
            ps_stat = ctx.enter_context(
                tc.tile_pool(name="ps_stat", bufs=6, space="PSUM")
            )
            ps_c1 = ctx.enter_context(tc.tile_pool(name="ps_c1", bufs=1, space="PSUM"))
            ps_c2 = ctx.enter_context(tc.tile_pool(name="ps_c2", bufs=1, space="PSUM"))

            # ---- constants ----
            ones_col = singles.tile([P, 1], f32)
            nc.vector.memset(ones_col, 1.0)
            ones_row = singles.tile([1, P], f32)
            nc.vector.memset(ones_row, 1.0)
            ones_scan = singles.tile([P, F2MAX], f32)
            nc.vector.memset(ones_scan, 1.0)
            lstrict_sb = singles.tile([P, P], f32)
            nc.sync.dma_start(lstrict_sb, lstrict)
            recip_sb = singles.tile([P, F2SUM], f32)
            nc.sync.dma_start(recip_sb, recip5)
            gamma_sb = singles.tile([P, NCH], f32)
            nc.sync.dma_start(gamma_sb, gamma_pc)
            beta_sb = singles.tile([P, NCH], f32)
            nc.sync.dma_start(beta_sb, beta_pc)
            eps_sb = singles.tile([P, 1], f32)
            nc.vector.memset(eps_sb, EPS)
            # running grand totals of (s1, r) over completed chunks, in SBUF
            gtot = singles.tile([1, 2], f32)

            xh = {}  # half index (cc, hh) -> tile [P, NCH, HB_cc]

            def load_half(cc, hh):
                hb = CHUNKS[cc] // 2
                t0 = OFFS[cc] + hh * hb
                xt = xhp.tile([P, NCH, hb], f32r, tag="xh", name=f"xh_{cc}_{hh}")
                nc.sync.dma_start(xt, xr[:, :, t0 : t0 + hb])
                xh[(cc, hh)] = xt

            def stats(cc):
                """Channel reductions for chunk cc.  Emits ACT squares, PE
                matmuls and ACT sq-row evacs inline; returns the DVE s1-row
                evac closures + the reshape-DMA closure for interleaving, and
                the scan-input tile."""
                ccs, kb, f2 = CHUNKS[cc], KBS[cc], F2S[cc]
                nkb_h = (ccs // 2) // kb
                srow = srowp.tile([33, ccs], f32, tag="srow", name=f"srow_{cc}")
                s1sq = s1sqp.tile([P, 2, f2], f32, tag="s1sq", name=f"s1sq_{cc}")
                s1_evacs = []
                for hh in range(2):
                    xt = xh[(cc, hh)]
                    xtf = xt.bitcast(f32)
                    for k in range(nkb_h):
                        kc = hh * nkb_h + k
                        ksl = slice(k * kb, (k + 1) * kb)
                        xsq = xsqp.tile(
                            [P, NCH, kb], f32r, tag="xsq", name=f"xsq_{cc}_{kc}"
                        )
                        for j in range(NCH):
                            nc.scalar.square(xsq[:, j, :], xtf[:, j, ksl])
                        s1p = ps_stat.tile([1, kb], f32, tag="st", name=f"s1p_{cc}_{kc}")
                        sqp = ps_stat.tile([1, kb], f32, tag="st", name=f"sqp_{cc}_{kc}")
                        for j in range(NCH):
                            nc.tensor.matmul(
                                s1p,
                                ones_col.bitcast(f32r),
                                xt[:, j, ksl],
                                start=(j == 0),
                                stop=(j == NCH - 1),
                            )
                        for j in range(NCH):
                            nc.tensor.matmul(
                                sqp,
                                ones_col.bitcast(f32r),
                                xsq[:, j, :],
                                start=(j == 0),
                                stop=(j == NCH - 1),
                            )
                        ksl2 = slice(kc * kb, (kc + 1) * kb)
                        nc.scalar.copy(srow[32:33, ksl2], sqp)
                        s1_evacs.append(
                            lambda ksl2=ksl2, s1p=s1p, srow=srow: nc.vector.tensor_copy(
                                srow[0:1, ksl2], s1p
                            )
                        )

                def reshape():
                    nc.sync.dma_start(s1sq[:, 0, :], srow[0:1, :])
                    nc.sync.dma_start(s1sq[:, 1, :], srow[32:33, :])

                return s1_evacs, reshape, s1sq

            def scan_steps(cc, s1sq):
                """Prefix-scan stats for chunk cc as a list of step closures
                (interleaved between normalize ops by the caller).
                Returns (steps, invnm tile)."""
                f2 = F2S[cc]
                s1c = s1sq[:, 0, :]
                sqc = s1sq[:, 1, :]
                rc = recip_sb[:, F2OFF[cc] : F2OFF[cc] + f2]
                osc = ones_scan[:, 0:f2]
                cum1 = statp.tile([P, f2], f32, tag="cum1", name=f"cum1_{cc}")
                carry1 = ps_c1.tile([P, 1], f32, tag="c1", name=f"c1_{cc}")
                carry1_sb = statp.tile([P, 1], f32, tag="cs1", name=f"cs1_{cc}")
                mean_c = statp.tile([P, f2], f32, tag="mean", name=f"mean_{cc}")
                u_c = statp.tile([P, f2], f32, tag="u", name=f"u_{cc}")
                cumr = statp.tile([P, f2], f32, tag="cumr", name=f"cumr_{cc}")
                carry2 = ps_c2.tile([P, 1], f32, tag="c2", name=f"c2_{cc}")
                carry2_sb = statp.tile([P, 1], f32, tag="cs2", name=f"cs2_{cc}")
                var_c = statp.tile([P, f2], f32, tag="var", name=f"var_{cc}")
                std_c = statp.tile([P, f2], f32, tag="std", name=f"std_{cc}")
                invnm = statp.tile([P, 2, f2], f32, tag="invnm", name=f"invnm_{cc}")
                last = cc == NCC - 1
                tot = (
                    None
                    if last
                    else ps_stat.tile([1, 2], f32, tag="st", name=f"tot_{cc}")
                )

                def s0():
                    nc.vector.tensor_tensor_scan(
                        cum1, osc, s1c, 0.0, Alu.mult, Alu.add
                    )
                    if cc > 0:
                        nc.tensor.matmul(
                            carry1, ones_row, gtot[0:1, 0:1], start=True, stop=False
                        )
                    nc.tensor.matmul(
                        carry1,
                        lstrict_sb,
                        cum1[:, f2 - 1 : f2],
                        start=(cc == 0),
                        stop=True,
                    )
                    if not last:
                        nc.tensor.matmul(
                            tot[0:1, 0:1],
                            ones_col,
                            cum1[:, f2 - 1 : f2],
                            start=True,
                            stop=True,
                        )

                def s1():
                    nc.vector.tensor_copy(carry1_sb, carry1)

                def s2():
                    nc.vector.scalar_tensor_tensor(
                        mean_c, cum1, carry1_sb, rc, Alu.add, Alu.mult
                    )

                def s3():
                    nc.vector.scalar_tensor_tensor(
                        u_c, mean_c, -float(C) / 2.0, s1c, Alu.mult, Alu.add
                    )
                    nc.vector.tensor_mul(u_c, mean_c, u_c)

                def s4():
                    nc.vector.scalar_tensor_tensor(
                        u_c, u_c, -2.0, sqc, Alu.mult, Alu.add
                    )

                def s5():
                    nc.vector.tensor_tensor_scan(
                        cumr, osc, u_c, 0.0, Alu.mult, Alu.add
                    )
                    if cc > 0:
                        nc.tensor.matmul(
                            carry2, ones_row, gtot[0:1, 1:2], start=True, stop=False
                        )
                    nc.tensor.matmul(
                        carry2,
                        lstrict_sb,
                        cumr[:, f2 - 1 : f2],
                        start=(cc == 0),
                        stop=True,
                    )
                    if not last:
                        nc.tensor.matmul(
                            tot[0:1, 1:2],
                            ones_col,
                            cumr[:, f2 - 1 : f2],
                            start=True,
                            stop=True,
                        )

                def s6():
                    nc.vector.tensor_copy(carry2_sb, carry2)

                def s7():
                    nc.vector.scalar_tensor_tensor(
                        var_c, cumr, carry2_sb, rc, Alu.add, Alu.mult
                    )
                    nc.scalar.activation(std_c, var_c, Act.Sqrt, bias=eps_sb)

                def s8():
                    if last:
                        return
                    if cc == 0:
                        nc.vector.tensor_copy(gtot, tot)
                    else:
                        tot_sb = statp.tile([1, 2], f32, tag="tsb", name=f"tsb_{cc}")
                        nc.vector.tensor_copy(tot_sb, tot)
                        nc.vector.tensor_add(gtot, gtot, tot_sb)

                def s9():
                    nc.vector.reciprocal(invnm[:, 0, :], std_c)

                def s10():
                    nc.vector.scalar_tensor_tensor(
                        invnm[:, 1, :], mean_c, -1.0, invnm[:, 0, :], Alu.mult, Alu.mult
                    )

                return [s0, s1, s2, s3, s4, s5, s6, s7, s8, s9, s10], invnm

            def make_bc(cc, hh, invnm, col):
                """inv (col 0) or nminv (col 1) row for half hh -> broadcast
                tile [P, HB].  Split per stat so the normalize mul can start
                as soon as inv lands, before nminv is even computed."""
                hb = CHUNKS[cc] // 2
                np_h = hb // F2S[cc]  # partitions per half in the scan layout
                brow = browp.tile([1, hb], f32, tag="brow", name=f"brow_{cc}_{hh}_{col}")
                psl = slice(np_h * hh, np_h * (hh + 1))
                nc.sync.dma_start(brow, invnm[psl, col, :])
                bc = bcp.tile([P, hb], f32, tag="bc", name=f"bc_{cc}_{hh}_{col}")
                nc.gpsimd.partition_broadcast(bc, brow)
                return bc

            def norm_ops(cc, bcs):
                """Normalize closures for chunk cc, in place in the x tiles.
                Fast path (gamma==1, beta==0): the channel-group dim is fused
                into one op via a stride-0 broadcast AP, 8 ops per chunk.
                General path: per-group mul + fused affine_then_add, 16 ops."""
                hb = CHUNKS[cc] // 2
                ops = []
                for hh in range(2):
                    xtf = xh[(cc, hh)].bitcast(f32)
                    bci, bcn = bcs[hh]
                    if trivial_affine:
                        for q in range(2):
                            qsl = slice(q * (hb // 2), (q + 1) * (hb // 2))
                            xq = xtf[:, :, qsl]
                            inv_b, _ = bass.broadcast_tensor_aps(
                                bci[:, None, qsl], xq
                            )
                            nm_b, _ = bass.broadcast_tensor_aps(
                                bcn[:, None, qsl], xq
                            )

                            def mul(xq=xq, inv_b=inv_b):
                                nc.vector.tensor_mul(xq, xq, inv_b)

                            def add(xq=xq, nm_b=nm_b):
                                nc.vector.tensor_add(xq, xq, nm_b)

                            ops.append(mul)
                            ops.append(add)
                    else:
                        for j in range(NCH):
                            xj = xtf[:, j, :]

                            def mul(xj=xj, bci=bci, j=j):
                                nc.vector.scalar_tensor_tensor(
                                    xj,
                                    xj,
                                    gamma_sb[:, j : j + 1],
                                    bci,
                                    Alu.mult,
                                    Alu.mult,
                                )

                            def add(xj=xj, bcn=bcn, j=j):
                                nc.vector.affine_then_add(
                                    xj,
                                    bcn,
                                    xj,
                                    scale=gamma_sb[:, j : j + 1],
                                    bias=beta_sb[:, j : j + 1],
                                )

                            ops.append(mul)
                            ops.append(add)
                return ops

            def store(cc, hh):
                hb = CHUNKS[cc] // 2
                t0 = OFFS[cc] + hh * hb
                xtf = xh[(cc, hh)].bitcast(f32)
                nc.sync.dma_start(out[:, :, t0 : t0 + hb], xtf)

            # ---- prologue: chunks 0,1 and half of 2 in flight; full
            # stats+scan+bc chain for chunk 0 (nothing to overlap with yet)
            halves = [(cc, hh) for cc in range(NCC) for hh in range(2)]
            nld = 0
            for _ in range(5):
                load_half(*halves[nld])
                nld += 1
            ev0, rs0, s1sq0 = stats(0)
            for e in ev0:
                e()
            rs0()
            steps0, invnm0 = scan_steps(0, s1sq0)
            for st in steps0:
                st()
            bcs = [
                (make_bc(0, hh, invnm0, 0), make_bc(0, hh, invnm0, 1))
                for hh in range(2)
            ]

            # ---- software-pipelined bodies: normalize/store chunk cc while
            # chunk cc+1 runs stats+scan and chunk cc+2 streams in
            for cc in range(NCC):
                for _ in range(2):
                    if nld < len(halves):
                        load_half(*halves[nld])
                        nld += 1
                N = norm_ops(cc, bcs)
                nh = len(N) // 2  # ops per half
                if cc + 1 < NCC:
                    evacs, reshape, s1sq_n = stats(cc + 1)
                    S, invnm_n = scan_steps(cc + 1, s1sq_n)
                    # interleave: s1 evacs ride the first normalize ops, the
                    # scan chain rides the rest so its serial latency hides
                    # under normalize throughput.
                    h0, h1 = N[:nh], N[nh:]
                    ne = max(1, (len(evacs) + len(h0) - 2) // max(1, len(h0) - 1))
                    ei = 0
                    for i, op in enumerate(h0):
                        op()
                        while ei < len(evacs) and ei < (i + 1) * ne:
                            evacs[ei]()
                            ei += 1
                    while ei < len(evacs):
                        evacs[ei]()
                        ei += 1
                    reshape()
                    # scan groups between the h1 normalize ops
                    bc_n = [[None, None], [None, None]]

                    def emit_inv():
                        bc_n[0][0] = make_bc(cc + 1, 0, invnm_n, 0)
                        bc_n[1][0] = make_bc(cc + 1, 1, invnm_n, 0)

                    def emit_nm():
                        bc_n[0][1] = make_bc(cc + 1, 0, invnm_n, 1)
                        bc_n[1][1] = make_bc(cc + 1, 1, invnm_n, 1)

                    groups = [
                        (S[0],),
                        (S[1], S[2]),
                        (S[3], S[4]),
                        (S[5],),
                        (S[6], S[7]),
                        (S[8], S[9]),
                        (emit_inv,),
                        (S[10], emit_nm),
                    ]
                    gi = 0
                    for i, op in enumerate(h1):
                        if i == len(h1) - 1:
                            break
                        op()
                        take = max(1, (len(groups) - gi) // (len(h1) - 1 - i))
                        for _ in range(take):
                            if gi < len(groups):
                                for st in groups[gi]:
                                    st()
                                gi += 1
                    while gi < len(groups):
                        for st in groups[gi]:
                            st()
                        gi += 1
                    store(cc, 0)
                    h1[-1]()
                    store(cc, 1)
                    bcs = [tuple(bc_n[0]), tuple(bc_n[1])]
                else:
                    for op in N[:nh]:
                        op()
                    store(cc, 0)
                    for op in N[nh:]:
                        op()
                    store(cc, 1)

    nc.finalize()
    return nc


def _make_consts():
    cols = []
    for i in range(NCC):
        f2 = F2S[i]
        t = OFFS[i] + np.arange(CHUNKS[i], dtype=np.float64).reshape(P, f2)
        cols.append(1.0 / (C * (t + 1.0)))
    recip5 = np.ascontiguousarray(np.concatenate(cols, axis=1).astype(np.float32))
    lstrict = np.triu(np.ones((P, P), dtype=np.float32), k=1)
    return lstrict, recip5


def _make_in_map(xb, gamma, beta):
    """Per-core input dict. xb: [C, T] fp32; gamma/beta: [C]."""
    lstrict, recip5 = _make_consts()
    return {
        "x": np.ascontiguousarray(xb.reshape(NCH, P, T).transpose(1, 0, 2)),
        "lstrict": lstrict,
        "recip5": recip5,
        "gamma_pc": np.ascontiguousarray(gamma.reshape(NCH, P).T),
        "beta_pc": np.ascontiguousarray(beta.reshape(NCH, P).T),
    }


def _from_out_layout(o):
    """Device out [P, NCH, T] -> [C, T]."""
    return np.ascontiguousarray(o.transpose(1, 0, 2).reshape(C, T))


def kernel(x, gamma, beta):
    from concourse import bass_utils

    x = np.ascontiguousarray(np.asarray(x, dtype=np.float32))
    gamma = np.asarray(gamma, dtype=np.float32).reshape(C)
    beta = np.asarray(beta, dtype=np.float32).reshape(C)

    trivial = bool(np.all(gamma == 1.0) and np.all(beta == 0.0))
    if trivial not in _PROGS:
        _PROGS[trivial] = _build_program(trivial_affine=trivial)
    prog = _PROGS[trivial]

    in_maps = [_make_in_map(x[b], gamma, beta) for b in range(B)]
    res = bass_utils.run_bass_kernel_spmd(prog, in_maps, core_ids=list(range(B)))
    return np.stack(
        [_from_out_layout(res.results[b]["out"]) for b in range(B)], axis=0
    )
